# revision 27
# baseline (speedup 1.0000x reference)
"""2-layer GCN encoder on 8 Trainium2 NeuronCores (Bass/Tile).

Math: with dis = deg^{-1/2} (self-loops included), the GCN layer
    out = relu(D^{-1/2} A D^{-1/2} (X W) + b)
separates as
    out[v] = relu(dis[v] * (sum_{e: dst=v} dis[src]*X[src]) @ W + b)
so the per-edge norm disappears and both weight matmuls commute out of the
edge aggregation.  Aggregation is done as binary-selection matmuls on the
TensorEngine over dma_gather'ed rows of the dis-prescaled feature table.

Design (v2):
 - One shared edge schedule for both layers: the layer-1 table xs is stored
   in gid (packed) order so gather indices/dst-slot streams are identical
   across layers.  chunk(src) = core(src)//2 gives 4 contiguous 26112-row
   table quarters, one per SWDGE queue, and keeps indices within int16.
 - Self-loops are excluded from the edge lists; the self term is added as a
   feature-major tensor_tensor add on the PSUM->SBUF copy (xsT for layer 1,
   a transposed-DMA read of the own shard for layer 2).
 - Nodes are packed into 8 cores x 102 groups x 128 slots such that every
   (group, chunk) cell holds <= 512 edges -> fixed 4 tiles per cell
   (TPC=4), ~20% fewer gather descriptors + matmuls than TPC=5.
 - Gather-call tile order is k-major so per-cell padding lands at the call
   tail where trailing -1 indices are trimmed by the Q7 descriptor
   generator (the serial bottleneck).
 - Per-batch epilogue is batched over the 6 groups in one PSUM tile
   ([128, 6, dout]) -> ~6x fewer DVE instructions.
"""

import numpy as np
import ml_dtypes

import concourse.bacc as bacc
import concourse.tile as tile
import concourse.mybir as mybir
from concourse.bass_utils import run_bass_kernel_spmd

# problem shapes (hardcoded per contract)
N = 100000
E = 1600000
IN_DIM, HID, OUT_DIM = 128, 128, 64

# schedule constants
P = 128            # partitions / tile edge count
NC_ = 8            # cores
G = 102            # groups per core
W = 6              # groups per batch
NB = 17            # batches per layer (W*NB == G)
TPC = 4            # tiles per (group, chunk) cell
NSEC = 4           # chunks: chunk(src) = core(src)//2
BT = W * NSEC * TPC            # tiles per batch = 96
CALL_T = W * TPC               # tiles per gather call = 24
CALL_I = CALL_T * P            # idxs per gather call = 3072
IDXW = CALL_I // 16            # wrapped idx cols per call = 192
NODES_PC = G * P               # padded nodes per core = 13056
GFULL = NC_ * NODES_PC         # gathered table rows = 104448
QROWS = GFULL // NSEC          # table rows per chunk quarter = 26112
CELL_CAP = TPC * P             # 512

BF16 = ml_dtypes.bfloat16

_compiled = None  # cache across calls


# ----------------------------------------------------------------- host side

def _pack_nodes(deg, deg_chunk_of):
    """Assign nodes to cores then to per-core groups.

    deg: total in-degree (self-loops excluded) per node, used for balance.
    deg_chunk_of(cores): callable returning [N, 4] in-degree split by
        chunk(src)=core(src)//2 once cores are assigned.

    Returns node_core, node_gabs, node_slot (shape [N]).
    """
    import heapq
    # --- stage 1: cores, balancing degree with count cap ---
    order = np.argsort(-deg, kind="stable")
    counts = np.zeros(NC_, np.int64)
    loads = np.zeros(NC_, np.float64)
    node_core = np.empty(N, np.int64)
    h = [(0.0, c) for c in range(NC_)]
    heapq.heapify(h)
    spill = []
    for n in order:
        while True:
            load, c = heapq.heappop(h)
            if counts[c] < NODES_PC:
                break
            spill.append((load, c))
        node_core[n] = c
        counts[c] += 1
        loads[c] = load + deg[n]
        heapq.heappush(h, (loads[c], c))
        for s in spill:
            heapq.heappush(h, s)
        spill.clear()

    # --- stage 2: groups within each core, capping per-cell edge counts ---
    dchunk = deg_chunk_of(node_core)  # [N, 4]
    node_gabs = np.empty(N, np.int64)
    node_slot = np.empty(N, np.int64)
    for c in range(NC_):
        nodes = np.where(node_core == c)[0]
        vecs = dchunk[nodes]
        tot = vecs.sum(axis=1)
        order = np.argsort(-tot, kind="stable")
        loads_g = np.zeros((G, NSEC), np.int64)
        counts_g = np.zeros(G, np.int64)
        for j in order:
            v = vecs[j]
            peak = (loads_g + v).max(axis=1)
            feas = (counts_g < P) & (peak <= CELL_CAP)
            if not feas.any():
                raise RuntimeError("group packing failed; raise G or cap")
            peak_f = np.where(feas, peak, 1 << 30)
            g = int(np.argmin(peak_f))
            node_gabs[nodes[j]] = g
            node_slot[nodes[j]] = counts_g[g]
            loads_g[g] += v
            counts_g[g] += 1
        if loads_g.max() > CELL_CAP:
            raise RuntimeError("cell overflow")
    return node_core, node_gabs, node_slot


def _build_schedule(qidx, chunk, ecore, egabs, eslot):
    """Shared-layer gather-index and dst-slot streams.

    qidx: per-edge row index within its chunk quarter (gid % QROWS).
    chunk: per-edge chunk (core(src)//2).
    ecore/egabs/eslot: dst placement per edge.

    Returns idx_dram [8, 128, NB*4*IDXW] int16 (wrapped, -1 tail padding)
    and dl_dram [8, 128, NB*BT] bf16 (dst slot per edge, 128 for padding).
    """
    cell = (ecore * G + egabs) * NSEC + chunk
    order = np.lexsort((qidx, cell))
    cell_s = cell[order]
    counts = np.bincount(cell, minlength=NC_ * G * NSEC)
    if counts.max() > CELL_CAP:
        raise RuntimeError(f"schedule overflow: {counts.max()} > {CELL_CAP}")
    starts = np.concatenate([[0], np.cumsum(counts)[:-1]])
    rank = np.arange(len(cell_s)) - np.repeat(starts, counts)
    ch = cell_s % NSEC
    gg = (cell_s // NSEC) % G
    cr = cell_s // (NSEC * G)
    batch = gg // W
    gslot = gg % W
    tile_k = rank // P
    pos = rank % P
    # k-major tile order within a call: tile = k*W + gslot
    T = batch * BT + ch * CALL_T + tile_k * W + gslot
    goff = cr * (NB * BT * P) + T * P + pos
    flat_idx = np.full(NC_ * NB * BT * P, -1, np.int64)
    flat_dl = np.full(NC_ * NB * BT * P, P, np.int16)
    flat_idx[goff] = qidx[order]
    flat_dl[goff] = eslot[order].astype(np.int16)
    # interior padding (before the last real edge of each call) must be a
    # valid row (0); only the contiguous call tail keeps -1 (Q7 trims it).
    fi = flat_idx.reshape(NC_ * NB * NSEC, CALL_I)
    for row in fi:
        nz = np.nonzero(row >= 0)[0]
        end = nz[-1] + 1 if len(nz) else 0
        row[:end][row[:end] < 0] = 0
    flat_idx = fi.reshape(-1).astype(np.int16)
    # wrapped idx layout per call: wrapped[p, s] = call[s*16 + p%16], x8
    fi = flat_idx.reshape(NC_, NB * NSEC, IDXW, 16)
    A = fi.transpose(0, 3, 1, 2).reshape(NC_, 16, NB * NSEC * IDXW)
    idx_dram = np.tile(A, (1, 8, 1))
    dl_dram = (
        flat_dl.reshape(NC_, NB * BT, P).transpose(0, 2, 1).astype(BF16)
    )
    return np.ascontiguousarray(idx_dram), np.ascontiguousarray(dl_dram)


def preprocess(x, edge_index):
    src = np.asarray(edge_index[0], dtype=np.int64)
    dst = np.asarray(edge_index[1], dtype=np.int64)
    deg = np.bincount(dst, minlength=N).astype(np.float64) + 1.0  # + self
    dis = (1.0 / np.sqrt(deg)).astype(np.float32)

    # drop self-loops from the gathered edge list (handled via direct add)
    keep = src != dst
    src_e = src[keep]
    dst_e = dst[keep]

    deg_ns = np.bincount(dst_e, minlength=N)  # non-self in-degree

    def deg_chunk_of(node_core):
        chunk = node_core[src_e] // 2
        dc = np.zeros((N, NSEC), np.int64)
        np.add.at(dc, (dst_e, chunk), 1)
        return dc

    node_core, node_gabs, node_slot = _pack_nodes(deg_ns, deg_chunk_of)
    gid = node_core * NODES_PC + node_gabs * P + node_slot

    xs = (np.asarray(x, np.float32) * dis[:, None]).astype(BF16)
    # gid-ordered table (padding rows zero)
    xs_pk = np.zeros((GFULL, IN_DIM), BF16)
    xs_pk[gid] = xs
    # per-core transposed shard [128 f, 13056 (g,slot)]
    xsT = np.ascontiguousarray(
        xs_pk.reshape(NC_, NODES_PC, IN_DIM).transpose(0, 2, 1)
    )

    chunk = node_core[src_e] // 2
    qidx = gid[src_e] % QROWS
    idx_dram, dl_dram = _build_schedule(
        qidx, chunk, node_core[dst_e], node_gabs[dst_e], node_slot[dst_e]
    )

    dis_sb = np.zeros((NC_, P, G), np.float32)
    dis_sb[node_core, node_slot, node_gabs] = dis

    return dict(
        xs_pk=xs_pk, xsT=xsT, idx=idx_dram, dl=dl_dram, dis_sb=dis_sb, gid=gid
    )


# --------------------------------------------------------------- device side

def build_program():
    f32 = mybir.dt.float32
    bf16 = mybir.dt.bfloat16
    i16 = mybir.dt.int16
    AO = mybir.AluOpType

    nc = bacc.Bacc("TRN2", target_bir_lowering=False, debug=False,
                   num_devices=NC_, num_swdge_queues=4,
                   dynamic_dma_scratch_size=32768)
    xs_d = nc.dram_tensor("xs_pk", [GFULL, IN_DIM], bf16, kind="ExternalInput")
    xsT_d = nc.dram_tensor("xsT", [P, NODES_PC], bf16, kind="ExternalInput")
    idx_d = nc.dram_tensor("idx", [P, NB * NSEC * IDXW], i16, kind="ExternalInput")
    dl_d = nc.dram_tensor("dl", [P, NB * BT], bf16, kind="ExternalInput")
    dis_d = nc.dram_tensor("dis", [P, G], f32, kind="ExternalInput")
    w1_d = nc.dram_tensor("w1", [IN_DIM, HID], bf16, kind="ExternalInput")
    w2_d = nc.dram_tensor("w2", [HID, OUT_DIM], bf16, kind="ExternalInput")
    b1_d = nc.dram_tensor("b1r", [P, HID], f32, kind="ExternalInput")
    b2_d = nc.dram_tensor("b2r", [P, OUT_DIM], f32, kind="ExternalInput")
    iota_d = nc.dram_tensor("iota", [P, P], bf16, kind="ExternalInput")
    out_d = nc.dram_tensor("out", [NODES_PC, OUT_DIM], f32, kind="ExternalOutput")

    with tile.TileContext(nc) as tc:
        with tc.tile_pool(name="const", bufs=1) as cpool, \
             tc.tile_pool(name="io", bufs=3) as iopool, \
             tc.tile_pool(name="msgp", bufs=3) as mpool, \
             tc.tile_pool(name="sp", bufs=2) as spool, \
             tc.tile_pool(name="epi", bufs=3) as epool, \
             tc.tile_pool(name="psag", bufs=2, space="PSUM") as psag, \
             tc.tile_pool(name="psep", bufs=2, space="PSUM") as psep, \
             tc.tile_pool(name="dram", bufs=1, space="DRAM") as dpool:

            w1s = cpool.tile([IN_DIM, HID], bf16)
            nc.sync.dma_start(out=w1s[:], in_=w1_d[:])
            w2s = cpool.tile([HID, OUT_DIM], bf16)
            nc.sync.dma_start(out=w2s[:], in_=w2_d[:])
            b1s = cpool.tile([P, HID], f32)
            nc.sync.dma_start(out=b1s[:], in_=b1_d[:])
            b2s = cpool.tile([P, OUT_DIM], f32)
            nc.sync.dma_start(out=b2s[:], in_=b2_d[:])
            dis_s = cpool.tile([P, G], f32)
            nc.sync.dma_start(out=dis_s[:], in_=dis_d[:])
            iota_s = cpool.tile([P, P], bf16)
            nc.sync.dma_start(out=iota_s[:], in_=iota_d[:])

            gshard = dpool.tile([NODES_PC, HID], bf16)
            gfull = dpool.tile([GFULL, HID], bf16, addr_space="Shared")

            def layer(tbl_d, wsb, dout, is_l1):
                for b in range(NB):
                    idx_t = iopool.tile([P, NSEC * IDXW], i16, tag="idx")
                    nc.sync.dma_start(
                        out=idx_t[:],
                        in_=idx_d[:, b * NSEC * IDXW:(b + 1) * NSEC * IDXW])
                    dl_t = iopool.tile([P, BT], bf16, tag="dl")
                    nc.sync.dma_start(
                        out=dl_t[:], in_=dl_d[:, b * BT:(b + 1) * BT])
                    # own-shard (transposed) rows for the self-loop term
                    selfT = epool.tile([P, W, P], bf16, tag="selfT")
                    if is_l1:
                        nc.sync.dma_start(
                            out=selfT[:],
                            in_=xsT_d[:, b * W * P:(b + 1) * W * P])
                    else:
                        nc.sync.dma_start_transpose(
                            out=selfT[:],
                            in_=gshard[b * W * P:(b + 1) * W * P, :])

                    msg = mpool.tile([P, BT, P], bf16, tag="msg")
                    if is_l1 and b < 3:
                        nc.vector.memset(msg[:], 0.0)
                    for c in range(NSEC):
                        nc.gpsimd.dma_gather(
                            out_ap=msg[:, c * CALL_T:(c + 1) * CALL_T, :],
                            in_ap=tbl_d[c * QROWS:(c + 1) * QROWS, :],
                            idxs_ap=idx_t[:, c * IDXW:(c + 1) * IDXW],
                            num_idxs=CALL_I,
                            num_idxs_reg=CALL_I,
                            elem_size=IN_DIM,
                            single_packet=False,
                            queue_num=c,
                        )
                    S3 = spool.tile([P, BT, P], bf16, tag="S3")
                    nc.vector.tensor_tensor(
                        out=S3[:],
                        in0=dl_t[:].unsqueeze(2).to_broadcast([P, BT, P]),
                        in1=iota_s[:].unsqueeze(1).to_broadcast([P, BT, P]),
                        op=AO.is_equal,
                    )
                    ps6 = psag.tile([P, W, P], f32, tag="agg")
                    for g in range(W):
                        for c in range(NSEC):
                            for k in range(TPC):
                                t = c * CALL_T + k * W + g
                                nc.tensor.matmul(
                                    out=ps6[:, g, :],
                                    lhsT=msg[:, t, :],
                                    rhs=S3[:, t, :],
                                    start=(c == 0 and k == 0),
                                    stop=(c == NSEC - 1 and k == TPC - 1),
                                )
                    # PSUM -> SBUF with fused self-loop add (feature-major)
                    aggT6 = epool.tile([P, W, P], bf16, tag="aggT6")
                    nc.vector.tensor_tensor(
                        out=aggT6[:], in0=ps6[:], in1=selfT[:], op=AO.add)
                    po6 = psep.tile([P, W, dout], f32, tag="po")
                    for g in range(W):
                        nc.tensor.matmul(
                            out=po6[:, g, :], lhsT=aggT6[:, g, :], rhs=wsb[:],
                            start=True, stop=True)
                    disbc = dis_s[:, b * W:(b + 1) * W].unsqueeze(2).to_broadcast(
                        [P, W, dout])
                    if is_l1:
                        u = epool.tile([P, W, dout], f32, tag="u")
                        nc.vector.tensor_tensor(
                            out=u[:], in0=po6[:], in1=disbc, op=AO.mult)
                        v = epool.tile([P, W, dout], f32, tag="v")
                        nc.vector.tensor_tensor(
                            out=v[:], in0=u[:],
                            in1=b1s[:].unsqueeze(1).to_broadcast([P, W, dout]),
                            op=AO.add)
                        r = epool.tile([P, W, dout], f32, tag="r")
                        nc.vector.tensor_scalar(
                            out=r[:], in0=v[:], scalar1=0.0, scalar2=None,
                            op0=AO.max)
                        gt = epool.tile([P, W, dout], bf16, tag="gt")
                        nc.vector.tensor_tensor(
                            out=gt[:], in0=r[:], in1=disbc, op=AO.mult)
                        nc.sync.dma_start(
                            out=gshard[:].rearrange(
                                "(g p) d -> p g d", p=P
                            )[:, b * W:(b + 1) * W, :],
                            in_=gt[:])
                    else:
                        u = epool.tile([P, W, dout], f32, tag="u")
                        nc.vector.tensor_tensor(
                            out=u[:], in0=po6[:], in1=disbc, op=AO.mult)
                        o = epool.tile([P, W, dout], f32, tag="o")
                        nc.vector.tensor_tensor(
                            out=o[:], in0=u[:],
                            in1=b2s[:].unsqueeze(1).to_broadcast([P, W, dout]),
                            op=AO.add)
                        nc.sync.dma_start(
                            out=out_d[:].rearrange(
                                "(g p) d -> p g d", p=P
                            )[:, b * W:(b + 1) * W, :],
                            in_=o[:])

            layer(xs_d, w1s, HID, True)

            nc.gpsimd.collective_compute(
                "AllGather",
                mybir.AluOpType.bypass,
                replica_groups=[list(range(NC_))],
                ins=[gshard.opt()],
                outs=[gfull.opt()],
            )

            layer(gfull, w2s, OUT_DIM, False)

    nc.compile()
    return nc


# ------------------------------------------------------------------- runner

def run(inputs, trace=False):
    global _compiled
    x = np.asarray(inputs["x"], np.float32)
    edge_index = np.asarray(inputs["edge_index"])
    W1 = np.asarray(inputs["W1"], np.float32)
    b1 = np.asarray(inputs["b1"], np.float32)
    W2 = np.asarray(inputs["W2"], np.float32)
    b2 = np.asarray(inputs["b2"], np.float32)

    pp = preprocess(x, edge_index)

    if _compiled is None:
        _compiled = build_program()
    nc = _compiled

    iota = np.broadcast_to(np.arange(P, dtype=np.float32), (P, P)).astype(BF16)
    b1r = np.broadcast_to(b1, (P, HID)).astype(np.float32)
    b2r = np.broadcast_to(b2, (P, OUT_DIM)).astype(np.float32)
    w1b = W1.astype(BF16)
    w2b = W2.astype(BF16)

    in_maps = []
    for c in range(NC_):
        in_maps.append({
            "xs_pk": pp["xs_pk"],
            "xsT": pp["xsT"][c],
            "idx": pp["idx"][c],
            "dl": pp["dl"][c],
            "dis": pp["dis_sb"][c],
            "w1": w1b,
            "w2": w2b,
            "b1r": np.ascontiguousarray(b1r),
            "b2r": np.ascontiguousarray(b2r),
            "iota": np.ascontiguousarray(iota),
        })

    res = run_bass_kernel_spmd(
        nc, in_maps, core_ids=list(range(NC_)), trace=trace
    )
    allf = np.concatenate([res.results[c]["out"] for c in range(NC_)], axis=0)
    out = allf[pp["gid"]].astype(np.float32)
    return out, res


def kernel(**inputs):
    out, _ = run(inputs, trace=False)
    return out


# revision 31
# speedup vs baseline: 1.0166x; 1.0166x over previous
"""2-layer GCN encoder on 8 Trainium2 NeuronCores (Bass/Tile).

Math: with dis = deg^{-1/2} (self-loops included), the GCN layer
    out = relu(D^{-1/2} A D^{-1/2} (X W) + b)
separates as
    out[v] = relu(dis[v] * (sum_{e: dst=v} dis[src]*X[src]) @ W + b)
so the per-edge norm disappears and both weight matmuls commute out of the
edge aggregation.  Aggregation is done as binary-selection matmuls on the
TensorEngine over dma_gather'ed rows of the dis-prescaled feature table.

Design (v2):
 - One shared edge schedule for both layers: the layer-1 table xs is stored
   in gid (packed) order so gather indices/dst-slot streams are identical
   across layers.  chunk(src) = core(src)//2 gives 4 contiguous 26112-row
   table quarters, one per SWDGE queue, and keeps indices within int16.
 - Self-loops are excluded from the edge lists; the self term is added as a
   feature-major tensor_tensor add on the PSUM->SBUF copy (xsT for layer 1,
   a transposed-DMA read of the own shard for layer 2).
 - Nodes are packed into 8 cores x 102 groups x 128 slots such that every
   (group, chunk) cell holds <= 512 edges -> fixed 4 tiles per cell
   (TPC=4), ~20% fewer gather descriptors + matmuls than TPC=5.
 - Gather-call tile order is k-major so per-cell padding lands at the call
   tail where trailing -1 indices are trimmed by the Q7 descriptor
   generator (the serial bottleneck).
 - Per-batch epilogue is batched over the 6 groups in one PSUM tile
   ([128, 6, dout]) -> ~6x fewer DVE instructions.
"""

import numpy as np
import ml_dtypes

import concourse.bacc as bacc
import concourse.tile as tile
import concourse.mybir as mybir
from concourse.bass_utils import run_bass_kernel_spmd

# problem shapes (hardcoded per contract)
N = 100000
E = 1600000
IN_DIM, HID, OUT_DIM = 128, 128, 64

# schedule constants
P = 128            # partitions / tile edge count
NC_ = 8            # cores
G = 102            # groups per core
W = 6              # groups per batch
NB = 17            # batches per layer (W*NB == G)
TPC = 4            # tiles per (group, chunk) cell
NSEC = 4           # chunks: chunk(src) = core(src)//2
BT = W * NSEC * TPC            # tiles per batch = 96
CALL_T = W * TPC               # tiles per gather call = 24
CALL_I = CALL_T * P            # idxs per gather call = 3072
IDXW = CALL_I // 16            # wrapped idx cols per call = 192
NODES_PC = G * P               # padded nodes per core = 13056
GFULL = NC_ * NODES_PC         # gathered table rows = 104448
QROWS = GFULL // NSEC          # table rows per chunk quarter = 26112
CELL_CAP = TPC * P             # 512

BF16 = ml_dtypes.bfloat16

_compiled = None  # cache across calls


# ----------------------------------------------------------------- host side

def _pack_nodes(deg, deg_chunk_of):
    """Assign nodes to cores then to per-core groups.

    deg: total in-degree (self-loops excluded) per node, used for balance.
    deg_chunk_of(cores): callable returning [N, 4] in-degree split by
        chunk(src)=core(src)//2 once cores are assigned.

    Returns node_core, node_gabs, node_slot (shape [N]).
    """
    import heapq
    # --- stage 1: cores, balancing degree with count cap ---
    order = np.argsort(-deg, kind="stable")
    counts = np.zeros(NC_, np.int64)
    loads = np.zeros(NC_, np.float64)
    node_core = np.empty(N, np.int64)
    h = [(0.0, c) for c in range(NC_)]
    heapq.heapify(h)
    spill = []
    for n in order:
        while True:
            load, c = heapq.heappop(h)
            if counts[c] < NODES_PC:
                break
            spill.append((load, c))
        node_core[n] = c
        counts[c] += 1
        loads[c] = load + deg[n]
        heapq.heappush(h, (loads[c], c))
        for s in spill:
            heapq.heappush(h, s)
        spill.clear()

    # --- stage 2: groups within each core, capping per-cell edge counts ---
    dchunk = deg_chunk_of(node_core)  # [N, 4]
    node_gabs = np.empty(N, np.int64)
    node_slot = np.empty(N, np.int64)
    for c in range(NC_):
        nodes = np.where(node_core == c)[0]
        vecs = dchunk[nodes]
        tot = vecs.sum(axis=1)
        order = np.argsort(-tot, kind="stable")
        loads_g = np.zeros((G, NSEC), np.int64)
        counts_g = np.zeros(G, np.int64)
        for j in order:
            v = vecs[j]
            peak = (loads_g + v).max(axis=1)
            feas = (counts_g < P) & (peak <= CELL_CAP)
            if not feas.any():
                raise RuntimeError("group packing failed; raise G or cap")
            peak_f = np.where(feas, peak, 1 << 30)
            g = int(np.argmin(peak_f))
            node_gabs[nodes[j]] = g
            node_slot[nodes[j]] = counts_g[g]
            loads_g[g] += v
            counts_g[g] += 1
        if loads_g.max() > CELL_CAP:
            raise RuntimeError("cell overflow")
    return node_core, node_gabs, node_slot


def _build_schedule(qidx, chunk, ecore, egabs, eslot):
    """Shared-layer gather-index and dst-slot streams.

    qidx: per-edge row index within its chunk quarter (gid % QROWS).
    chunk: per-edge chunk (core(src)//2).
    ecore/egabs/eslot: dst placement per edge.

    Returns idx_dram [8, 128, NB*4*IDXW] int16 (wrapped, -1 tail padding)
    and dl_dram [8, 128, NB*BT] bf16 (dst slot per edge, 128 for padding).
    """
    cell = (ecore * G + egabs) * NSEC + chunk
    order = np.lexsort((qidx, cell))
    cell_s = cell[order]
    counts = np.bincount(cell, minlength=NC_ * G * NSEC)
    if counts.max() > CELL_CAP:
        raise RuntimeError(f"schedule overflow: {counts.max()} > {CELL_CAP}")
    starts = np.concatenate([[0], np.cumsum(counts)[:-1]])
    rank = np.arange(len(cell_s)) - np.repeat(starts, counts)
    ch = cell_s % NSEC
    gg = (cell_s // NSEC) % G
    cr = cell_s // (NSEC * G)
    batch = gg // W
    gslot = gg % W
    tile_k = rank // P
    pos = rank % P
    # k-major tile order within a call: tile = k*W + gslot
    T = batch * BT + ch * CALL_T + tile_k * W + gslot
    goff = cr * (NB * BT * P) + T * P + pos
    flat_idx = np.full(NC_ * NB * BT * P, -1, np.int64)
    flat_dl = np.full(NC_ * NB * BT * P, P, np.int16)
    flat_idx[goff] = qidx[order]
    flat_dl[goff] = eslot[order].astype(np.int16)
    # interior padding (before the last real edge of each call) must be a
    # valid row (0); only the contiguous call tail keeps -1 (Q7 trims it).
    # First 3 batches pad fully with 0 so msg buffers never hold stale
    # data on their first use (replaces an SBUF memset).
    fi = flat_idx.reshape(NC_ * NB * NSEC, CALL_I)
    for ri, row in enumerate(fi):
        b = (ri % (NB * NSEC)) // NSEC
        if b < 3:
            row[row < 0] = 0
            continue
        nz = np.nonzero(row >= 0)[0]
        end = nz[-1] + 1 if len(nz) else 0
        row[:end][row[:end] < 0] = 0
    flat_idx = fi.reshape(-1).astype(np.int16)
    # wrapped idx layout per call: wrapped[p, s] = call[s*16 + p%16], x8
    fi = flat_idx.reshape(NC_, NB * NSEC, IDXW, 16)
    A = fi.transpose(0, 3, 1, 2).reshape(NC_, 16, NB * NSEC * IDXW)
    idx_dram = np.tile(A, (1, 8, 1))
    dl_dram = (
        flat_dl.reshape(NC_, NB * BT, P).transpose(0, 2, 1).astype(BF16)
    )
    return np.ascontiguousarray(idx_dram), np.ascontiguousarray(dl_dram)


def preprocess(x, edge_index):
    src = np.asarray(edge_index[0], dtype=np.int64)
    dst = np.asarray(edge_index[1], dtype=np.int64)
    deg = np.bincount(dst, minlength=N).astype(np.float64) + 1.0  # + self
    dis = (1.0 / np.sqrt(deg)).astype(np.float32)

    # drop self-loops from the gathered edge list (handled via direct add)
    keep = src != dst
    src_e = src[keep]
    dst_e = dst[keep]

    deg_ns = np.bincount(dst_e, minlength=N)  # non-self in-degree

    def deg_chunk_of(node_core):
        chunk = node_core[src_e] // 2
        dc = np.zeros((N, NSEC), np.int64)
        np.add.at(dc, (dst_e, chunk), 1)
        return dc

    node_core, node_gabs, node_slot = _pack_nodes(deg_ns, deg_chunk_of)
    gid = node_core * NODES_PC + node_gabs * P + node_slot

    xs = (np.asarray(x, np.float32) * dis[:, None]).astype(BF16)
    # gid-ordered table (padding rows zero)
    xs_pk = np.zeros((GFULL, IN_DIM), BF16)
    xs_pk[gid] = xs
    # per-core transposed shard [128 f, 13056 (g,slot)]
    xsT = np.ascontiguousarray(
        xs_pk.reshape(NC_, NODES_PC, IN_DIM).transpose(0, 2, 1)
    )

    chunk = node_core[src_e] // 2
    qidx = gid[src_e] % QROWS
    idx_dram, dl_dram = _build_schedule(
        qidx, chunk, node_core[dst_e], node_gabs[dst_e], node_slot[dst_e]
    )

    dis_sb = np.zeros((NC_, P, G), np.float32)
    dis_sb[node_core, node_slot, node_gabs] = dis

    return dict(
        xs_pk=xs_pk, xsT=xsT, idx=idx_dram, dl=dl_dram, dis_sb=dis_sb, gid=gid
    )


# --------------------------------------------------------------- device side

def build_program():
    f32 = mybir.dt.float32
    bf16 = mybir.dt.bfloat16
    i16 = mybir.dt.int16
    AO = mybir.AluOpType

    nc = bacc.Bacc("TRN2", target_bir_lowering=False, debug=False,
                   num_devices=NC_, num_swdge_queues=4,
                   dynamic_dma_scratch_size=32768)
    xs_d = nc.dram_tensor("xs_pk", [GFULL, IN_DIM], bf16, kind="ExternalInput")
    xsT_d = nc.dram_tensor("xsT", [P, NODES_PC], bf16, kind="ExternalInput")
    idx_d = nc.dram_tensor("idx", [P, NB * NSEC * IDXW], i16, kind="ExternalInput")
    dl_d = nc.dram_tensor("dl", [P, NB * BT], bf16, kind="ExternalInput")
    dis_d = nc.dram_tensor("dis", [P, G], f32, kind="ExternalInput")
    w1_d = nc.dram_tensor("w1", [IN_DIM, HID], bf16, kind="ExternalInput")
    w2_d = nc.dram_tensor("w2", [HID, OUT_DIM], bf16, kind="ExternalInput")
    b1_d = nc.dram_tensor("b1r", [P, HID], f32, kind="ExternalInput")
    b2_d = nc.dram_tensor("b2r", [P, OUT_DIM], f32, kind="ExternalInput")
    iota_d = nc.dram_tensor("iota", [P, P], bf16, kind="ExternalInput")
    out_d = nc.dram_tensor("out", [NODES_PC, OUT_DIM], f32, kind="ExternalOutput")

    with tile.TileContext(nc) as tc:
        with tc.tile_pool(name="const", bufs=1) as cpool, \
             tc.tile_pool(name="io", bufs=4) as iopool, \
             tc.tile_pool(name="msgp", bufs=3) as mpool, \
             tc.tile_pool(name="sp", bufs=2) as spool, \
             tc.tile_pool(name="epi", bufs=3) as epool, \
             tc.tile_pool(name="psag", bufs=2, space="PSUM") as psag, \
             tc.tile_pool(name="psep", bufs=2, space="PSUM") as psep, \
             tc.tile_pool(name="dram", bufs=1, space="DRAM") as dpool:

            w1s = cpool.tile([IN_DIM, HID], bf16)
            nc.sync.dma_start(out=w1s[:], in_=w1_d[:])
            w2s = cpool.tile([HID, OUT_DIM], bf16)
            nc.sync.dma_start(out=w2s[:], in_=w2_d[:])
            b1s = cpool.tile([P, HID], f32)
            nc.sync.dma_start(out=b1s[:], in_=b1_d[:])
            b2s = cpool.tile([P, OUT_DIM], f32)
            nc.sync.dma_start(out=b2s[:], in_=b2_d[:])
            dis_s = cpool.tile([P, G], f32)
            nc.sync.dma_start(out=dis_s[:], in_=dis_d[:])
            iota_s = cpool.tile([P, P], bf16)
            nc.sync.dma_start(out=iota_s[:], in_=iota_d[:])

            gshard = dpool.tile([NODES_PC, HID], bf16)
            gfull = dpool.tile([GFULL, HID], bf16)

            def layer(tbl_d, wsb, dout, is_l1):
                for b in range(NB):
                    idx_t = iopool.tile([P, NSEC * IDXW], i16, tag="idx")
                    nc.sync.dma_start(
                        out=idx_t[:],
                        in_=idx_d[:, b * NSEC * IDXW:(b + 1) * NSEC * IDXW])
                    dl_t = iopool.tile([P, BT], bf16, tag="dl")
                    nc.sync.dma_start(
                        out=dl_t[:], in_=dl_d[:, b * BT:(b + 1) * BT])
                    # own-shard (transposed) rows for the self-loop term
                    selfT = epool.tile([P, W, P], bf16, tag="selfT")
                    if is_l1:
                        nc.sync.dma_start(
                            out=selfT[:],
                            in_=xsT_d[:, b * W * P:(b + 1) * W * P])
                    else:
                        nc.sync.dma_start_transpose(
                            out=selfT[:],
                            in_=gshard[b * W * P:(b + 1) * W * P, :])

                    msg = mpool.tile([P, BT, P], bf16, tag="msg")
                    for c in range(NSEC):
                        nc.gpsimd.dma_gather(
                            out_ap=msg[:, c * CALL_T:(c + 1) * CALL_T, :],
                            in_ap=tbl_d[c * QROWS:(c + 1) * QROWS, :],
                            idxs_ap=idx_t[:, c * IDXW:(c + 1) * IDXW],
                            num_idxs=CALL_I,
                            num_idxs_reg=CALL_I,
                            elem_size=IN_DIM,
                            single_packet=False,
                            queue_num=c,
                        )
                    S3 = spool.tile([P, BT, P], bf16, tag="S3")
                    nc.vector.tensor_tensor(
                        out=S3[:],
                        in0=dl_t[:].unsqueeze(2).to_broadcast([P, BT, P]),
                        in1=iota_s[:].unsqueeze(1).to_broadcast([P, BT, P]),
                        op=AO.is_equal,
                    )
                    ps6 = psag.tile([P, W, P], f32, tag="agg")
                    for g in range(W):
                        for c in range(NSEC):
                            for k in range(TPC):
                                t = c * CALL_T + k * W + g
                                nc.tensor.matmul(
                                    out=ps6[:, g, :],
                                    lhsT=msg[:, t, :],
                                    rhs=S3[:, t, :],
                                    start=(c == 0 and k == 0),
                                    stop=(c == NSEC - 1 and k == TPC - 1),
                                )
                    # PSUM -> SBUF with fused self-loop add (feature-major)
                    aggT6 = epool.tile([P, W, P], bf16, tag="aggT6")
                    nc.vector.tensor_tensor(
                        out=aggT6[:], in0=ps6[:], in1=selfT[:], op=AO.add)
                    po6 = psep.tile([P, W, dout], f32, tag="po")
                    for g in range(W):
                        nc.tensor.matmul(
                            out=po6[:, g, :], lhsT=aggT6[:, g, :], rhs=wsb[:],
                            start=True, stop=True)
                    disbc = dis_s[:, b * W:(b + 1) * W].unsqueeze(2).to_broadcast(
                        [P, W, dout])
                    if is_l1:
                        u = epool.tile([P, W, dout], f32, tag="u")
                        nc.vector.tensor_tensor(
                            out=u[:], in0=po6[:], in1=disbc, op=AO.mult)
                        v = epool.tile([P, W, dout], f32, tag="v")
                        nc.vector.tensor_tensor(
                            out=v[:], in0=u[:],
                            in1=b1s[:].unsqueeze(1).to_broadcast([P, W, dout]),
                            op=AO.add)
                        r = epool.tile([P, W, dout], f32, tag="r")
                        nc.vector.tensor_scalar(
                            out=r[:], in0=v[:], scalar1=0.0, scalar2=None,
                            op0=AO.max)
                        gt = epool.tile([P, W, dout], bf16, tag="gt")
                        nc.vector.tensor_tensor(
                            out=gt[:], in0=r[:], in1=disbc, op=AO.mult)
                        nc.sync.dma_start(
                            out=gshard[:].rearrange(
                                "(g p) d -> p g d", p=P
                            )[:, b * W:(b + 1) * W, :],
                            in_=gt[:])
                    else:
                        u = epool.tile([P, W, dout], f32, tag="u")
                        nc.vector.tensor_tensor(
                            out=u[:], in0=po6[:], in1=disbc, op=AO.mult)
                        o = epool.tile([P, W, dout], f32, tag="o")
                        nc.vector.tensor_tensor(
                            out=o[:], in0=u[:],
                            in1=b2s[:].unsqueeze(1).to_broadcast([P, W, dout]),
                            op=AO.add)
                        nc.sync.dma_start(
                            out=out_d[:].rearrange(
                                "(g p) d -> p g d", p=P
                            )[:, b * W:(b + 1) * W, :],
                            in_=o[:])

            layer(xs_d, w1s, HID, True)

            nc.gpsimd.collective_compute(
                "AllGather",
                mybir.AluOpType.bypass,
                replica_groups=[list(range(NC_))],
                ins=[gshard.opt()],
                outs=[gfull.opt()],
            )

            layer(gfull, w2s, OUT_DIM, False)

    nc.compile()
    return nc


# ------------------------------------------------------------------- runner

def run(inputs, trace=False):
    global _compiled
    x = np.asarray(inputs["x"], np.float32)
    edge_index = np.asarray(inputs["edge_index"])
    W1 = np.asarray(inputs["W1"], np.float32)
    b1 = np.asarray(inputs["b1"], np.float32)
    W2 = np.asarray(inputs["W2"], np.float32)
    b2 = np.asarray(inputs["b2"], np.float32)

    pp = preprocess(x, edge_index)

    if _compiled is None:
        _compiled = build_program()
    nc = _compiled

    iota = np.broadcast_to(np.arange(P, dtype=np.float32), (P, P)).astype(BF16)
    b1r = np.broadcast_to(b1, (P, HID)).astype(np.float32)
    b2r = np.broadcast_to(b2, (P, OUT_DIM)).astype(np.float32)
    w1b = W1.astype(BF16)
    w2b = W2.astype(BF16)

    in_maps = []
    for c in range(NC_):
        in_maps.append({
            "xs_pk": pp["xs_pk"],
            "xsT": pp["xsT"][c],
            "idx": pp["idx"][c],
            "dl": pp["dl"][c],
            "dis": pp["dis_sb"][c],
            "w1": w1b,
            "w2": w2b,
            "b1r": np.ascontiguousarray(b1r),
            "b2r": np.ascontiguousarray(b2r),
            "iota": np.ascontiguousarray(iota),
        })

    res = run_bass_kernel_spmd(
        nc, in_maps, core_ids=list(range(NC_)), trace=trace
    )
    allf = np.concatenate([res.results[c]["out"] for c in range(NC_)], axis=0)
    out = allf[pp["gid"]].astype(np.float32)
    return out, res


def kernel(**inputs):
    out, _ = run(inputs, trace=False)
    return out
